# revision 40
# baseline (speedup 1.0000x reference)
"""Trainium2 Bass kernel for nn_DifferentiableStack (B=1024, L=1024, D=128, STACK=32).

Key simplification: in the reference, the push/pop gates broadcast over all
stack slots identically and the initial stack is zero, so every slot holds the
same vector. The output top-of-stack is just the scalar linear recurrence
    h_t = h_{t-1} * (1 - o_t) + x_t * p_t,      out = h_{L-1}
which unrolls to a weighted reduction over time:
    out[b,:] = sum_t x[b,t,:] * w[b,t],   w[b,t] = p[b,t] * prod_{s>t}(1 - o[b,s]).

Truncation: with uniform(0,1) pop gates the suffix product decays about
2^-1.44 per step, so weights for all but the last 128 timesteps fall below
fp32 rounding of the O(1) output (the fp32 reference itself cannot see them).
kernel() proves the bound on the actual gate values (host-side, cheap) and
falls back to a full-length variant if it ever fails.

Sharding: pure data parallel, batch dim 1024 -> 8 cores x 128 rows.

Per-core program (Tile framework):
  Phase A (few us, overlapped with Phase B's DMA): load the kept gate tail
    [128b, LK]; a = 1-o; suffix products via log2(LK) shifted elementwise
    multiplies on a [128, 2*LK] ones-padded buffer; w = p * (shifted suffix);
    TensorE transpose -> w_T [128t, tk, 128b].
  Phase B: x tiles of the kept t-blocks DMA'd as [128t, 8b, 128d] (512B
    contiguous runs, t on partitions); per (b, t-block) one matmul with the
    weight column as the 4-byte-self-loading stationary operand and the x
    tile moving:  psum[1, 128d] (+)= w_col.T @ x_tile, accumulated over kept
    t-blocks in PSUM partition 0 at per-b bank offsets; per 8-b group one DVE
    eviction [1, 1024] into an SBUF output row.
  Output: out_row [1, 128*128] (b-major) -> DRAM; host reshapes to [128, 128].
"""

import numpy as np

B_TOTAL, L, D = 1024, 1024, 128
N_CORES = 8
B_LOC = B_TOTAL // N_CORES  # 128

_NC_CACHE = {}

# build configuration (overridable for experiments)
CONFIG = {
    "BC": 8,
    "x_bufs": 12,
    # NOTE: alternating HWDGE rings ("sync", "scalar") intermittently wedges
    # the device (NRT_EXEC_UNIT_UNRECOVERABLE); single-ring sync is stable.
    "dma_engines": ("sync",),
    "gpsimd_identity": True,
    "swap": True,
    # The pop gates are uniform(0,1), so suffix products decay ~2^-1.44/step;
    # weights for t < L-128 are below fp32 rounding of the output with
    # overwhelming probability. kernel() verifies this bound on the actual
    # inputs and falls back to the full-length variant if violated.
    "tb_keep": 1,
}


def _build_nc(L=1024, BC=16, x_bufs=6, loop_k=None, dma_engines=("sync", "scalar"),
              gpsimd_identity=True, skip_matmul=False, skip_xdma=False, fp32r=False,
              mm_transpose=False, swap=False, tb_keep=None, pair64=False):
    import concourse.bacc as bacc
    import concourse.mybir as mybir
    import concourse.tile as tile
    from concourse import masks

    F32 = mybir.dt.float32
    B, Dd = 128, 128
    TB = L // 128
    if tb_keep is None:
        tb_keep = TB
    TB0 = TB - tb_keep          # first kept t-block
    LK = tb_keep * 128          # kept timesteps (tail)
    if pair64:
        LK = 64                 # keep last 64 steps; 2 batch rows share the
                                # 128 partitions of each matmul (block-diag w)
    STEPS = (LK - 1).bit_length()
    assert 1 << STEPS == LK

    nc = bacc.Bacc("TRN2", target_bir_lowering=False, debug=False, num_devices=8)
    x_dt = mybir.dt.float32r if fp32r else F32
    x_dram = nc.dram_tensor("x", [B, L, Dd], x_dt, kind="ExternalInput")
    pg_dram = nc.dram_tensor("pg", [B, L], F32, kind="ExternalInput")
    og_dram = nc.dram_tensor("og", [B, L], F32, kind="ExternalInput")
    if swap:
        out_dram = nc.dram_tensor("out", [1, B * Dd], F32, kind="ExternalOutput")
    else:
        out_dram = nc.dram_tensor("out", [Dd, B], F32, kind="ExternalOutput")
    ident_dram = None
    if not gpsimd_identity:
        ident_dram = nc.dram_tensor("ident", [128, 128], F32, kind="ExternalInput")

    with tile.TileContext(nc) as tc:
        with (
            tc.tile_pool(name="const", bufs=1) as cpool,
            tc.tile_pool(name="gates", bufs=1 if swap else 2) as gpool,
            tc.tile_pool(name="xtiles", bufs=x_bufs) as xpool,
            tc.tile_pool(name="pst", bufs=2, space="PSUM") as ppool,
            tc.tile_pool(name="psmm", bufs=2, space="PSUM") as mmpool,
            tc.tile_pool(name="outp", bufs=1) as opool,
        ):
            ident = cpool.tile([128, 128], F32)
            if gpsimd_identity:
                masks.make_identity(nc, ident[:])
            else:
                # avoid gpsimd entirely: identity comes from host as input
                nc.sync.dma_start(ident[:], ident_dram[:])

            def body(_iv=None):
                og_sb = gpool.tile([B, LK], F32, tag="og")
                pg_sb = gpool.tile([B, LK], F32, tag="pg")
                nc.sync.dma_start(og_sb[:], og_dram[:, L - LK : L])
                nc.sync.dma_start(pg_sb[:], pg_dram[:, L - LK : L])

                A0 = gpool.tile([B, 2 * LK], F32, tag="A0")
                A1 = gpool.tile([B, 2 * LK], F32, tag="A1")
                nc.vector.memset(A0[:, LK : 2 * LK], 1.0)
                nc.vector.memset(A1[:, LK : 2 * LK], 1.0)
                nc.vector.tensor_scalar(
                    A0[:, 0:LK], og_sb[:], -1.0, 1.0,
                    op0=mybir.AluOpType.mult, op1=mybir.AluOpType.add,
                )
                cur, nxt = A0, A1
                for k in range(STEPS):
                    s = 1 << k
                    nc.vector.tensor_tensor(
                        nxt[:, 0:LK], cur[:, 0:LK], cur[:, s : s + LK],
                        op=mybir.AluOpType.mult,
                    )
                    cur, nxt = nxt, cur
                w_bt = gpool.tile([B, LK], F32, tag="wbt")
                nc.vector.tensor_tensor(
                    w_bt[:], pg_sb[:], cur[:, 1 : LK + 1], op=mybir.AluOpType.mult
                )
                if pair64:
                    # Build W_shift [128b, 128] with row b's 64 weights at
                    # column offset parity(b)*64, zeros elsewhere; transposing
                    # gives w2 [(h,t), b] whose column pair (2c, 2c+1) is the
                    # block-diagonal stationary for batch pair c.
                    W_shift = gpool.tile([B, 128], F32, tag="wshift")
                    nc.vector.memset(W_shift[:], 0.0)
                    nc.vector.tensor_copy(W_shift[0:128:2, 0:64], w_bt[0:128:2, :])
                    nc.vector.tensor_copy(W_shift[1:128:2, 64:128], w_bt[1:128:2, :])
                    pt = ppool.tile([128, 128], F32, tag="pt")
                    nc.tensor.transpose(pt[:], W_shift[:], ident[:])
                    w2 = gpool.tile([128, 128], F32, tag="w2")
                    nc.vector.tensor_copy(w2[:], pt[:])

                    out_row = opool.tile([1, B * Dd], F32, tag="acc")
                    T0 = L - LK
                    n_groups = (B // 2) // BC  # BC pairs per group
                    for ci in range(n_groups):
                        pg_ps = mmpool.tile([2, BC * Dd], F32, tag="mm")
                        xt = xpool.tile([128, BC, Dd], x_dt, tag="xt")
                        src = x_dram[
                            ci * 2 * BC : (ci + 1) * 2 * BC, T0:L, :
                        ].rearrange("(j h) t d -> h t j d", h=2)
                        dst = xt[:].rearrange("(h t) j d -> h t j d", h=2)
                        nc.sync.dma_start(dst, src)
                        for j in range(BC):
                            c = ci * BC + j
                            nc.tensor.matmul(
                                pg_ps[0:2, j * Dd : (j + 1) * Dd],
                                w2[:, 2 * c : 2 * c + 2],
                                xt[:, j, :],
                                skip_group_check=True,
                            )
                        # evict parity rows into the b-major output row
                        seg = out_row[
                            0:1, ci * BC * 2 * Dd : (ci + 1) * BC * 2 * Dd
                        ].rearrange("p (j h d) -> p h j d", h=2, d=Dd)
                        src0 = pg_ps[0:1, :].rearrange("p (j d) -> p j d", d=Dd)
                        src1 = pg_ps[1:2, :].rearrange("p (j d) -> p j d", d=Dd)
                        nc.vector.tensor_copy(seg[:, 0, :, :], src0)
                        nc.scalar.copy(seg[:, 1, :, :], src1)
                    nc.sync.dma_start(out_dram[:], out_row[:])
                    return

                w_T = gpool.tile(
                    [128, tb_keep, B], mybir.dt.float32r if fp32r else F32, tag="wT"
                )
                for tk in range(tb_keep):
                    pt = ppool.tile([128, 128], F32, tag="pt")
                    nc.tensor.transpose(
                        pt[:], w_bt[:, tk * 128 : (tk + 1) * 128], ident[:]
                    )
                    nc.vector.tensor_copy(w_T[:, tk, :], pt[:])

                if swap:
                    # stationary = w column [128t, 1]; moving = x tile [128t, 128d];
                    # out [1, 128d] on PSUM partition 0, accumulated over t-blocks.
                    out_row = opool.tile([1, B * Dd], F32, tag="acc")
                    n_chunks = B // BC
                    for ci in range(n_chunks):
                        pg_ps = mmpool.tile([1, BC * Dd], F32, tag="mm")
                        for tk in range(tb_keep):
                            tb = TB0 + tk
                            xt = xpool.tile([128, BC, Dd], x_dt, tag="xt")
                            src = x_dram[
                                ci * BC : (ci + 1) * BC, tb * 128 : (tb + 1) * 128, :
                            ].transpose([1, 0, 2])
                            eng = getattr(
                                nc,
                                dma_engines[(ci * tb_keep + tk) % len(dma_engines)],
                            )
                            eng.dma_start(xt[:], src)
                            for j in range(BC):
                                b = ci * BC + j
                                lhsT = w_T[:, tk, b : b + 1]
                                rhs = xt[:, j, :]
                                nc.tensor.matmul(
                                    pg_ps[0:1, j * Dd : (j + 1) * Dd],
                                    lhsT,
                                    rhs,
                                    start=(tk == 0),
                                    stop=(tk == tb_keep - 1),
                                    skip_group_check=True,
                                )
                        # alternate eviction engine: keep DVE free for phase A
                        # and spread PSUM reads across DVE and ACT
                        dst = out_row[0:1, ci * BC * Dd : (ci + 1) * BC * Dd]
                        if ci % 2 == 0:
                            nc.vector.tensor_copy(dst, pg_ps[:])
                        else:
                            nc.scalar.copy(dst, pg_ps[:])
                    nc.sync.dma_start(out_dram[:], out_row[:])
                    return

                acc = opool.tile([Dd, B], F32, tag="acc")
                n_chunks = B // BC
                for tk in range(tb_keep):
                    tb = TB0 + tk
                    mm = mmpool.tile([Dd, B], F32, tag="mm")
                    for ci in range(n_chunks):
                        xt = xpool.tile([128, BC, Dd], x_dt, tag="xt")
                        src = x_dram[
                            ci * BC : (ci + 1) * BC, tb * 128 : (tb + 1) * 128, :
                        ].transpose([1, 0, 2])
                        eng = getattr(
                            nc, dma_engines[(tk * n_chunks + ci) % len(dma_engines)]
                        )
                        if not skip_xdma:
                            eng.dma_start(xt[:], src)
                        else:
                            # minimal write so Tile sees the tile allocated
                            eng.dma_start(xt[:, 0:1, :], src[:, 0:1, :])
                        if not skip_matmul:
                            for j in range(BC):
                                b = ci * BC + j
                                lhsT = xt[:, j, :]
                                rhs = w_T[:, tk, b : b + 1]
                                if fp32r:
                                    lhsT = lhsT.bitcast(mybir.dt.float32r)
                                    rhs = rhs.bitcast(mybir.dt.float32r)
                                nc.tensor.matmul(
                                    mm[:, b : b + 1], lhsT, rhs,
                                    is_transpose=True if mm_transpose else None,
                                )
                    if skip_matmul:
                        continue
                    if tk == 0:
                        nc.vector.tensor_copy(acc[:], mm[:])
                    else:
                        nc.vector.tensor_tensor(
                            acc[:], acc[:], mm[:], op=mybir.AluOpType.add
                        )
                if skip_matmul:
                    # keep the output written: dump w_T instead of acc
                    nc.vector.tensor_copy(acc[:], w_T[:, 0, :])
                nc.sync.dma_start(out_dram[:], acc[:])

            if loop_k is None:
                body()
            else:
                with tc.For_i(0, loop_k, 1) as iv:
                    body(iv)

    nc.compile()
    return nc


def get_nc(loop_k=None, tb_keep_override=None):
    cfg = dict(CONFIG)
    if tb_keep_override == "full":
        cfg["tb_keep"] = None
    key = (loop_k, tuple(sorted(cfg.items())))
    if key not in _NC_CACHE:
        _NC_CACHE[key] = _build_nc(L=L, loop_k=loop_k, **cfg)
    return _NC_CACHE[key]


def make_in_maps(x, push_gate, pop_gate):
    pg = np.ascontiguousarray(push_gate.reshape(B_TOTAL, L))
    og = np.ascontiguousarray(pop_gate.reshape(B_TOTAL, L))
    maps = [
        {
            "x": x[c * B_LOC : (c + 1) * B_LOC],
            "pg": pg[c * B_LOC : (c + 1) * B_LOC],
            "og": og[c * B_LOC : (c + 1) * B_LOC],
        }
        for c in range(N_CORES)
    ]
    if not CONFIG["gpsimd_identity"]:
        eye = np.eye(128, dtype=np.float32)
        for m in maps:
            m["ident"] = eye
    return maps


def assemble_out(results):
    # full output is [B_TOTAL, D]; per core "out" is [D, B_LOC], or
    # [1, B_LOC*D] in b-major order for the swap variant
    if CONFIG.get("swap"):
        return np.concatenate(
            [np.asarray(results[c]["out"]).reshape(B_LOC, D) for c in range(N_CORES)],
            axis=0,
        )
    return np.concatenate(
        [np.asarray(results[c]["out"]).T for c in range(N_CORES)], axis=0
    )


def _truncation_safe(og_2d, lk):
    """True if dropping timesteps t < L-lk cannot affect the fp32 output.

    Every dropped term's weight is bounded by prod_{s in kept range}(1-o_s);
    if that product is < 2^-30 for every batch row, dropped contributions are
    far below fp32 rounding of the O(1) output.
    """
    tail = 1.0 - og_2d[:, L - lk :].astype(np.float64)
    with np.errstate(divide="ignore"):
        lg = np.log2(np.maximum(tail, 0.0))
    return float(lg.sum(axis=1).max()) < -30.0


def kernel(x, push_gate, pop_gate):
    from concourse.bass_utils import run_bass_kernel_spmd

    x = np.ascontiguousarray(np.asarray(x, dtype=np.float32))
    pg = np.asarray(push_gate, dtype=np.float32)
    og = np.asarray(pop_gate, dtype=np.float32)

    tbk = CONFIG.get("tb_keep")
    if tbk is not None and tbk * 128 < L and not _truncation_safe(
        og.reshape(B_TOTAL, L), tbk * 128
    ):
        # pathological gates: fall back to the full-length kernel
        nc = get_nc(tb_keep_override="full")
    else:
        nc = get_nc()
    in_maps = make_in_maps(x, pg, og)
    res = run_bass_kernel_spmd(nc, in_maps, list(range(N_CORES)))
    return assemble_out(res.results).astype(np.float32)


# revision 44
# speedup vs baseline: 1.1116x; 1.1116x over previous
"""Trainium2 Bass kernel for nn_DifferentiableStack (B=1024, L=1024, D=128, STACK=32).

Key simplification: in the reference, the push/pop gates broadcast over all
stack slots identically and the initial stack is zero, so every slot holds the
same vector. The output top-of-stack is just the scalar linear recurrence
    h_t = h_{t-1} * (1 - o_t) + x_t * p_t,      out = h_{L-1}
which unrolls to a weighted reduction over time:
    out[b,:] = sum_t x[b,t,:] * w[b,t],   w[b,t] = p[b,t] * prod_{s>t}(1 - o[b,s]).

Truncation: with uniform(0,1) pop gates the suffix product decays about
2^-1.44 per step, so weights for all but the last 128 timesteps fall below
fp32 rounding of the O(1) output (the fp32 reference itself cannot see them).
kernel() proves the bound on the actual gate values (host-side, cheap) and
falls back to a full-length variant if it ever fails.

Sharding: pure data parallel, batch dim 1024 -> 8 cores x 128 rows.

Per-core program (Tile framework):
  Phase A (few us, overlapped with Phase B's DMA): load the kept gate tail
    [128b, LK]; a = 1-o; suffix products via log2(LK) shifted elementwise
    multiplies on a [128, 2*LK] ones-padded buffer; w = p * (shifted suffix);
    TensorE transpose -> w_T [128t, tk, 128b].
  Phase B: x tiles of the kept t-blocks DMA'd as [128t, 8b, 128d] (512B
    contiguous runs, t on partitions); per (b, t-block) one matmul with the
    weight column as the 4-byte-self-loading stationary operand and the x
    tile moving:  psum[1, 128d] (+)= w_col.T @ x_tile, accumulated over kept
    t-blocks in PSUM partition 0 at per-b bank offsets; per 8-b group one DVE
    eviction [1, 1024] into an SBUF output row.
  Output: out_row [1, 128*128] (b-major) -> DRAM; host reshapes to [128, 128].
"""

import numpy as np

B_TOTAL, L, D = 1024, 1024, 128
N_CORES = 8
B_LOC = B_TOTAL // N_CORES  # 128

_NC_CACHE = {}

# build configuration (overridable for experiments)
CONFIG = {
    "BC": 8,
    "x_bufs": 12,
    # NOTE: alternating HWDGE rings ("sync", "scalar") intermittently wedges
    # the device (NRT_EXEC_UNIT_UNRECOVERABLE); single-ring sync is stable.
    "dma_engines": ("sync",),
    "gpsimd_identity": True,
    "swap": True,
    # The pop gates are uniform(0,1), so suffix products decay ~2^-1.44/step;
    # weights for t < L-128 are below fp32 rounding of the output with
    # overwhelming probability. kernel() verifies this bound on the actual
    # inputs and falls back to the full-length variant if violated.
    "tb_keep": 1,
}


def _build_nc(L=1024, BC=16, x_bufs=6, loop_k=None, dma_engines=("sync", "scalar"),
              gpsimd_identity=True, skip_matmul=False, skip_xdma=False, fp32r=False,
              mm_transpose=False, swap=False, tb_keep=None, pair64=False):
    import concourse.bacc as bacc
    import concourse.mybir as mybir
    import concourse.tile as tile
    from concourse import masks

    F32 = mybir.dt.float32
    B, Dd = 128, 128
    TB = L // 128
    if tb_keep is None:
        tb_keep = TB
    TB0 = TB - tb_keep          # first kept t-block
    LK = tb_keep * 128          # kept timesteps (tail)
    if pair64:
        LK = 64                 # keep last 64 steps; 2 batch rows share the
                                # 128 partitions of each matmul (block-diag w)
    STEPS = (LK - 1).bit_length()
    assert 1 << STEPS == LK

    nc = bacc.Bacc("TRN2", target_bir_lowering=False, debug=False, num_devices=8)
    x_dt = mybir.dt.float32r if fp32r else F32
    x_dram = nc.dram_tensor("x", [B, L, Dd], x_dt, kind="ExternalInput")
    pg_dram = nc.dram_tensor("pg", [B, L], F32, kind="ExternalInput")
    og_dram = nc.dram_tensor("og", [B, L], F32, kind="ExternalInput")
    if pair64:
        # row h holds parity-h outputs, c-major: out[2c+h, d] = out_dram[h, c*128+d]
        out_dram = nc.dram_tensor("out", [2, B * Dd // 2], F32, kind="ExternalOutput")
    elif swap:
        out_dram = nc.dram_tensor("out", [1, B * Dd], F32, kind="ExternalOutput")
    else:
        out_dram = nc.dram_tensor("out", [Dd, B], F32, kind="ExternalOutput")
    ident_dram = None
    if not gpsimd_identity:
        ident_dram = nc.dram_tensor("ident", [128, 128], F32, kind="ExternalInput")
    pmask_dram = None
    if pair64:
        # col 0: 1.0 on even partitions; col 1: 1.0 on odd partitions
        pmask_dram = nc.dram_tensor("pmask", [128, 2], F32, kind="ExternalInput")

    with tile.TileContext(nc) as tc:
        with (
            tc.tile_pool(name="const", bufs=1) as cpool,
            tc.tile_pool(name="gates", bufs=1 if swap else 2) as gpool,
            tc.tile_pool(name="xtiles", bufs=x_bufs) as xpool,
            tc.tile_pool(name="pst", bufs=2, space="PSUM") as ppool,
            tc.tile_pool(name="psmm", bufs=2, space="PSUM") as mmpool,
            tc.tile_pool(name="outp", bufs=1) as opool,
        ):
            ident = cpool.tile([128, 128], F32)
            if gpsimd_identity:
                masks.make_identity(nc, ident[:])
            else:
                # avoid gpsimd entirely: identity comes from host as input
                nc.sync.dma_start(ident[:], ident_dram[:])

            def body(_iv=None):
                og_sb = gpool.tile([B, LK], F32, tag="og")
                pg_sb = gpool.tile([B, LK], F32, tag="pg")
                nc.sync.dma_start(og_sb[:], og_dram[:, L - LK : L])
                nc.sync.dma_start(pg_sb[:], pg_dram[:, L - LK : L])

                A0 = gpool.tile([B, 2 * LK], F32, tag="A0")
                A1 = gpool.tile([B, 2 * LK], F32, tag="A1")
                nc.vector.memset(A0[:, LK : 2 * LK], 1.0)
                nc.vector.memset(A1[:, LK : 2 * LK], 1.0)
                nc.vector.tensor_scalar(
                    A0[:, 0:LK], og_sb[:], -1.0, 1.0,
                    op0=mybir.AluOpType.mult, op1=mybir.AluOpType.add,
                )
                cur, nxt = A0, A1
                for k in range(STEPS):
                    s = 1 << k
                    nc.vector.tensor_tensor(
                        nxt[:, 0:LK], cur[:, 0:LK], cur[:, s : s + LK],
                        op=mybir.AluOpType.mult,
                    )
                    cur, nxt = nxt, cur
                w_bt = gpool.tile([B, LK], F32, tag="wbt")
                nc.vector.tensor_tensor(
                    w_bt[:], pg_sb[:], cur[:, 1 : LK + 1], op=mybir.AluOpType.mult
                )
                if pair64:
                    # Build W_shift [128b, 128] with row b's 64 weights at
                    # column offset parity(b)*64, zeros elsewhere; transposing
                    # gives w2 [(h,t), b] whose column pair (2c, 2c+1) is the
                    # block-diagonal stationary for batch pair c.
                    W_shift = gpool.tile([B, 128], F32, tag="wshift")
                    pmask = gpool.tile([128, 2], F32, tag="pmask")
                    nc.sync.dma_start(pmask[:], pmask_dram[:])
                    nc.vector.tensor_scalar(
                        W_shift[:, 0:64], w_bt[:], pmask[:, 0:1], None,
                        op0=mybir.AluOpType.mult,
                    )
                    nc.vector.tensor_scalar(
                        W_shift[:, 64:128], w_bt[:], pmask[:, 1:2], None,
                        op0=mybir.AluOpType.mult,
                    )
                    pt = ppool.tile([128, 128], F32, tag="pt")
                    nc.tensor.transpose(pt[:], W_shift[:], ident[:])
                    w2 = gpool.tile([128, 128], F32, tag="w2")
                    nc.vector.tensor_copy(w2[:], pt[:])

                    out_row = opool.tile([2, B * Dd // 2], F32, tag="acc")
                    T0 = L - LK
                    n_groups = (B // 2) // BC  # BC pairs per group
                    for ci in range(n_groups):
                        pg_ps = mmpool.tile([2, BC * Dd], F32, tag="mm")
                        xt = xpool.tile([128, BC, Dd], x_dt, tag="xt")
                        # one 64KB DMA per batch pair: src (2, 64, 128) against
                        # dst [128, 128] stays within the 3-dim AP balance cap
                        for j in range(BC):
                            c = ci * BC + j
                            nc.sync.dma_start(
                                xt[:, j, :], x_dram[2 * c : 2 * c + 2, T0:L, :]
                            )
                        for j in range(BC):
                            c = ci * BC + j
                            nc.tensor.matmul(
                                pg_ps[0:2, j * Dd : (j + 1) * Dd],
                                w2[:, 2 * c : 2 * c + 2],
                                xt[:, j, :],
                                skip_group_check=True,
                            )
                        # partition-aligned eviction: psum rows 0/1 -> out_row
                        # rows 0/1 (parity kept separate; host interleaves)
                        nc.vector.tensor_copy(
                            out_row[0:2, ci * BC * Dd : (ci + 1) * BC * Dd],
                            pg_ps[:],
                        )
                    nc.sync.dma_start(out_dram[:], out_row[:])
                    return

                w_T = gpool.tile(
                    [128, tb_keep, B], mybir.dt.float32r if fp32r else F32, tag="wT"
                )
                for tk in range(tb_keep):
                    pt = ppool.tile([128, 128], F32, tag="pt")
                    nc.tensor.transpose(
                        pt[:], w_bt[:, tk * 128 : (tk + 1) * 128], ident[:]
                    )
                    nc.vector.tensor_copy(w_T[:, tk, :], pt[:])

                if swap:
                    # stationary = w column [128t, 1]; moving = x tile [128t, 128d];
                    # out [1, 128d] on PSUM partition 0, accumulated over t-blocks.
                    out_row = opool.tile([1, B * Dd], F32, tag="acc")
                    n_chunks = B // BC
                    for ci in range(n_chunks):
                        pg_ps = mmpool.tile([1, BC * Dd], F32, tag="mm")
                        for tk in range(tb_keep):
                            tb = TB0 + tk
                            xt = xpool.tile([128, BC, Dd], x_dt, tag="xt")
                            src = x_dram[
                                ci * BC : (ci + 1) * BC, tb * 128 : (tb + 1) * 128, :
                            ].transpose([1, 0, 2])
                            eng = getattr(
                                nc,
                                dma_engines[(ci * tb_keep + tk) % len(dma_engines)],
                            )
                            eng.dma_start(xt[:], src)
                            for j in range(BC):
                                b = ci * BC + j
                                lhsT = w_T[:, tk, b : b + 1]
                                rhs = xt[:, j, :]
                                nc.tensor.matmul(
                                    pg_ps[0:1, j * Dd : (j + 1) * Dd],
                                    lhsT,
                                    rhs,
                                    start=(tk == 0),
                                    stop=(tk == tb_keep - 1),
                                    skip_group_check=True,
                                )
                        # alternate eviction engine: keep DVE free for phase A
                        # and spread PSUM reads across DVE and ACT
                        dst = out_row[0:1, ci * BC * Dd : (ci + 1) * BC * Dd]
                        if ci % 2 == 0:
                            nc.vector.tensor_copy(dst, pg_ps[:])
                        else:
                            nc.scalar.copy(dst, pg_ps[:])
                    nc.sync.dma_start(out_dram[:], out_row[:])
                    return

                acc = opool.tile([Dd, B], F32, tag="acc")
                n_chunks = B // BC
                for tk in range(tb_keep):
                    tb = TB0 + tk
                    mm = mmpool.tile([Dd, B], F32, tag="mm")
                    for ci in range(n_chunks):
                        xt = xpool.tile([128, BC, Dd], x_dt, tag="xt")
                        src = x_dram[
                            ci * BC : (ci + 1) * BC, tb * 128 : (tb + 1) * 128, :
                        ].transpose([1, 0, 2])
                        eng = getattr(
                            nc, dma_engines[(tk * n_chunks + ci) % len(dma_engines)]
                        )
                        if not skip_xdma:
                            eng.dma_start(xt[:], src)
                        else:
                            # minimal write so Tile sees the tile allocated
                            eng.dma_start(xt[:, 0:1, :], src[:, 0:1, :])
                        if not skip_matmul:
                            for j in range(BC):
                                b = ci * BC + j
                                lhsT = xt[:, j, :]
                                rhs = w_T[:, tk, b : b + 1]
                                if fp32r:
                                    lhsT = lhsT.bitcast(mybir.dt.float32r)
                                    rhs = rhs.bitcast(mybir.dt.float32r)
                                nc.tensor.matmul(
                                    mm[:, b : b + 1], lhsT, rhs,
                                    is_transpose=True if mm_transpose else None,
                                )
                    if skip_matmul:
                        continue
                    if tk == 0:
                        nc.vector.tensor_copy(acc[:], mm[:])
                    else:
                        nc.vector.tensor_tensor(
                            acc[:], acc[:], mm[:], op=mybir.AluOpType.add
                        )
                if skip_matmul:
                    # keep the output written: dump w_T instead of acc
                    nc.vector.tensor_copy(acc[:], w_T[:, 0, :])
                nc.sync.dma_start(out_dram[:], acc[:])

            if loop_k is None:
                body()
            else:
                with tc.For_i(0, loop_k, 1) as iv:
                    body(iv)

    nc.compile()
    return nc


def get_nc(loop_k=None, tb_keep_override=None):
    cfg = dict(CONFIG)
    if tb_keep_override == "full":
        cfg["tb_keep"] = None
    key = (loop_k, tuple(sorted(cfg.items())))
    if key not in _NC_CACHE:
        _NC_CACHE[key] = _build_nc(L=L, loop_k=loop_k, **cfg)
    return _NC_CACHE[key]


def make_in_maps(x, push_gate, pop_gate):
    pg = np.ascontiguousarray(push_gate.reshape(B_TOTAL, L))
    og = np.ascontiguousarray(pop_gate.reshape(B_TOTAL, L))
    maps = [
        {
            "x": x[c * B_LOC : (c + 1) * B_LOC],
            "pg": pg[c * B_LOC : (c + 1) * B_LOC],
            "og": og[c * B_LOC : (c + 1) * B_LOC],
        }
        for c in range(N_CORES)
    ]
    if not CONFIG["gpsimd_identity"]:
        eye = np.eye(128, dtype=np.float32)
        for m in maps:
            m["ident"] = eye
    if CONFIG.get("pair64"):
        pm = np.zeros((128, 2), np.float32)
        pm[0::2, 0] = 1.0
        pm[1::2, 1] = 1.0
        for m in maps:
            m["pmask"] = pm
    return maps


def assemble_out(results):
    # full output is [B_TOTAL, D]; per core "out" is [D, B_LOC], or
    # [1, B_LOC*D] in b-major order for the swap variant
    if CONFIG.get("pair64"):
        outs = []
        for c in range(N_CORES):
            o = np.asarray(results[c]["out"]).reshape(2, B_LOC // 2, D)
            outs.append(np.transpose(o, (1, 0, 2)).reshape(B_LOC, D))
        return np.concatenate(outs, axis=0)
    if CONFIG.get("swap"):
        return np.concatenate(
            [np.asarray(results[c]["out"]).reshape(B_LOC, D) for c in range(N_CORES)],
            axis=0,
        )
    return np.concatenate(
        [np.asarray(results[c]["out"]).T for c in range(N_CORES)], axis=0
    )


def _truncation_safe(og_2d, lk):
    """True if dropping timesteps t < L-lk cannot affect the fp32 output.

    Every dropped term's weight is bounded by prod_{s in kept range}(1-o_s);
    if that product is < 2^-30 for every batch row, dropped contributions are
    far below fp32 rounding of the O(1) output.
    """
    tail = 1.0 - og_2d[:, L - lk :].astype(np.float64)
    with np.errstate(divide="ignore"):
        lg = np.log2(np.maximum(tail, 0.0))
    return float(lg.sum(axis=1).max()) < -30.0


def kernel(x, push_gate, pop_gate):
    from concourse.bass_utils import run_bass_kernel_spmd

    x = np.ascontiguousarray(np.asarray(x, dtype=np.float32))
    pg = np.asarray(push_gate, dtype=np.float32)
    og = np.asarray(pop_gate, dtype=np.float32)

    tbk = CONFIG.get("tb_keep")
    lk = 64 if CONFIG.get("pair64") else (tbk * 128 if tbk is not None else L)
    if lk < L and not _truncation_safe(og.reshape(B_TOTAL, L), lk):
        # pathological gates: fall back to the full-length kernel
        nc = get_nc(tb_keep_override="full")
    else:
        nc = get_nc()
    in_maps = make_in_maps(x, pg, og)
    res = run_bass_kernel_spmd(nc, in_maps, list(range(N_CORES)))
    return assemble_out(res.results).astype(np.float32)
